# revision 10
# baseline (speedup 1.0000x reference)
"""Trainium2 Bass kernel for nn_ConnectionC2G (GNN cross-attention message passing).

Math (per batch b):
    K = Wk @ img + bk            [32, L]   (img = image reshaped [256, L], L = 4096)
    V = Wv @ img + bv            [32, L]
    Qt = (Wq @ graph^T + bq)/s   [32, N]   (s = sqrt(32); scale folded into Wq, bq)
    S^T[l, n] = sum_o K[o,l] Qt[o,n]       (attention scores, transposed layout)
    softmax over n-axis of the ORIGINAL layout == per-l-row softmax in S^T layout
    message[o, n] = sum_l ((V[o,l]+bv)/den[l]) * exp(S^T[l,n])
    out^T = graph^T + Wc @ message + bc    [32, N]

Key tricks:
  - scores lie in [-2.6, 2.7] for this problem so exp() never overflows ->
    no max-subtraction pass.
  - HAM warm-up: the PE clock gate sits at 1.2 GHz until ~3.4us of continuous
    matmul activity.  The prologue runs the Q projections back-to-back as soon
    as graphT lands so the whole main loop runs at the 2.4 GHz warm clock.
  - exp is split across two engines: chunks A (1536 cols) and B (1024) go
    through ScalarE's table exp (accum_out gives their denominator parts),
    chunk C (1536) is computed on the DVE with the 2^y bit trick:
    bf16bits(e^s) ~= int16(s * 128*log2(e) + 127*128).  The C-chunk partial
    denominator rides along as accum_out of the bf16 copy pass.
  - 1/den and the V bias are folded into the per-tile [128, 32] vts tile, not
    the big [L, N] matrix.
  - message accumulates across all 32 l-tiles into 2 persistent PSUM banks
    using tile_position column strips; the strips are unpacked with ONE
    [128, 1024] DVE copy and consumed by Wc matmuls via tile_position row
    offsets.  The residual (graph^T) is added on the PE with an identity
    matmul; bc is folded into the PSUM->SBUF copies' bias.
  - sharding: data-parallel over batch, 1 batch per NeuronCore (8 cores).
"""

import numpy as np
import ml_dtypes

import concourse.bass as bass
import concourse.bacc as bacc
import concourse.tile as tile
from concourse import mybir
from concourse.bass_utils import run_bass_kernel_spmd

F32 = mybir.dt.float32
BF16 = mybir.dt.bfloat16
I16 = mybir.dt.int16
AF = mybir.ActivationFunctionType
OP = mybir.AluOpType

B = 8
N = 4096          # graph nodes
GC = 32           # graph channels
C = 256           # image channels
L = 4096          # image pixels (64*64)
LT = 128          # l-tile rows (partition dim of S^T tiles)
NLT = L // LT     # 32 l-tiles
NB = 512          # matmul moving-dim block
NNB = N // NB     # 8 n-blocks

# exp chunks: A, B on ScalarE (table exp + accum), C on DVE (bit-trick exp)
CA0, CA1 = 0, 1536
CB0, CB1 = 1536, 2560
CC0, CC1 = 2560, 4096
CCW = CC1 - CC0

# fast-exp constants: bf16 bits of 2^y = y*128 + 127*128, y = s*log2(e)
FX_SCALE = 128.0 * 1.4426950408889634
FX_BIAS = 127.0 * 128.0

FAST_EXP = True          # chunk C on DVE bit-trick exp (False: ScalarE exp)

TRACE = False            # test.py sets kernel.TRACE = True for profiling
LAST_RESULT = None       # test.py reads exec_time_ns from here

_NC_CACHE = {}


def _pair_ap(t, j):
    """AP covering cols [j*NB, (j+1)*NB) of BOTH channel halves of img."""
    base = t[:, j * NB:(j + 1) * NB]
    ap = list(base.ap)
    return bass.AP(tensor=base.tensor, offset=base.offset,
                   ap=[ap[0], [L, 2]] + ap[1:])


def build_kernel():
    nc = bacc.Bacc("TRN2")

    img_d = nc.dram_tensor("img", [128, 2 * L], BF16, kind="ExternalInput")
    graphTb_d = nc.dram_tensor("graphTb", [GC, N], BF16, kind="ExternalInput")
    # bf16 pack: [:,0:32] WkT rows 0:128 | [:,32:64] WkT rows 128:256
    #            [:,64:96] WvT rows 0:128 | [:,96:128] WvT rows 128:256
    #            [:,128:160] WcT tiled x4 | [0:32,160:192] WqT*s
    #            [0:32,192:224] I32 identity
    wkv_d = nc.dram_tensor("wkv", [128, 224], BF16, kind="ExternalInput")
    # f32 pack: [:,0] bq*s | [:,1] bk | [:,2] bc ; row 0 cols 8:40 = bv
    wq_d = nc.dram_tensor("wq", [GC, 40], F32, kind="ExternalInput")
    out_d = nc.dram_tensor("outT", [GC, N], F32, kind="ExternalOutput")

    with tile.TileContext(nc) as tc:
        with tc.tile_pool(name="persist", bufs=1) as persist:
            img = persist.tile([128, 2 * L], BF16, tag="img")
            graphTb = persist.tile([GC, N], BF16, tag="graphTb")
            wkv = persist.tile([128, 224], BF16, tag="wkv")
            wq = persist.tile([GC, 40], F32, tag="wq")
            bv_bcast = persist.tile([128, GC], F32, tag="bv_bcast")
            K_sb = persist.tile([GC, N], BF16, tag="K_sb")
            Qt = persist.tile([GC, N], BF16, tag="Qt")
            Vt_raw = persist.tile([128, NLT * GC], BF16, tag="Vt_raw")
            msg_sb = persist.tile([GC, N], BF16, tag="msg_sb")
            outT = persist.tile([GC, N], F32, tag="outT")

            # small tensors first (unblock Q projections), then the image as
            # 8 block-pairs spread over the two HW DGE queues (+1 on gpsimd)
            nc.scalar.dma_start(out=wkv[:], in_=wkv_d[:])
            nc.scalar.dma_start(out=wq[:], in_=wq_d[:])
            bv_row = wq_d[0:1, 8:40]
            nc.scalar.dma_start(
                out=bv_bcast[:],
                in_=bass.AP(tensor=bv_row.tensor, offset=bv_row.offset,
                            ap=[[0, 128]] + list(bv_row.ap[1:])))
            nc.scalar.dma_start(out=graphTb[:], in_=graphTb_d[:])
            # 4 col-windows x 2 halves as plain 2D DMAs; sync carries half 0,
            # scalar half 1, so window w (K blocks 2w, 2w+1) lands early.
            WW = 1024
            for w in range(4):
                cs = slice(w * WW, (w + 1) * WW)
                nc.sync.dma_start(out=img[:, cs], in_=img_d[:, cs])
                cs1 = slice(L + w * WW, L + (w + 1) * WW)
                nc.scalar.dma_start(out=img[:, cs1], in_=img_d[:, cs1])

            bq = wq[:, 0:1]
            bk = wq[:, 1:2]
            bc = wq[:, 2:3]

            # ---- prologue ------------------------------------------------
            with (
                tc.tile_pool(name="proj_psum", bufs=4,
                             space=bass.MemorySpace.PSUM) as pp,
                tc.tile_pool(name="vt_psum", bufs=2,
                             space=bass.MemorySpace.PSUM) as vtp,
            ):
                # Q projections first: 8 dense matmuls warm the PE HAM gate
                for j in range(NNB):
                    blk = slice(j * NB, (j + 1) * NB)
                    qp = pp.tile([GC, NB], F32, tag="proj")
                    nc.tensor.matmul(qp[:], wkv[0:32, 160:192], graphTb[:, blk],
                                     start=True, stop=True)
                    nc.scalar.activation(out=Qt[:, blk], in_=qp[:],
                                         func=AF.Identity, bias=bq, scale=1.0)
                # per image block: K projection + V^T l-tiles (chases DMA)
                for j in range(NNB):
                    blk = slice(j * NB, (j + 1) * NB)
                    kp = pp.tile([GC, NB], F32, tag="proj")
                    nc.tensor.matmul(kp[:], wkv[:, 0:32], img[:, blk],
                                     start=True, stop=False)
                    nc.tensor.matmul(kp[:], wkv[:, 32:64],
                                     img[:, L + j * NB:L + (j + 1) * NB],
                                     start=False, stop=True)
                    nc.vector.tensor_scalar_add(K_sb[:, blk], kp[:], bk)
                    # V^T tiles: each in its own full PSUM bank (matmul psum
                    # writes must start bank-aligned); copies split DVE/ScalarE
                    for i in range(4):
                        lt = 4 * j + i
                        vt = vtp.tile([128, NB], F32, tag="vt")
                        nc.tensor.matmul(vt[:, 0:GC],
                                         img[:, lt * LT:(lt + 1) * LT],
                                         wkv[:, 64:96], start=True, stop=False)
                        nc.tensor.matmul(vt[:, 0:GC],
                                         img[:, L + lt * LT:L + (lt + 1) * LT],
                                         wkv[:, 96:128], start=False, stop=True)
                        dst = Vt_raw[:, lt * GC:(lt + 1) * GC]
                        if lt % 2 == 0:
                            nc.vector.tensor_copy(dst, vt[:, 0:GC])
                        else:
                            nc.scalar.copy(dst, vt[:, 0:GC])

            # ---- main loop: scores -> exp -> message ---------------------
            with (
                tc.tile_pool(name="s_psum", bufs=2,
                             space=bass.MemorySpace.PSUM) as sp,
                tc.tile_pool(name="msg_psum", bufs=1,
                             space=bass.MemorySpace.PSUM) as mp,
                tc.tile_pool(name="e_pool", bufs=3) as ep,
                tc.tile_pool(name="ec_pool", bufs=2) as ecp,
                tc.tile_pool(name="stat", bufs=8) as stp,
            ):
                msg_ps = mp.tile([128, 1024], F32, tag="msg")
                prev = None  # (vts, e_t) of tile lt-1, msg emitted one behind

                def emit_msg(lt, vts, e_t):
                    for j in range(NNB):
                        cg = 32 * (j % 4)
                        hb = (j // 4) * NB
                        nc.tensor.matmul(
                            msg_ps[cg:cg + 32, hb:hb + NB],
                            vts[:], e_t[:, j * NB:(j + 1) * NB],
                            start=(lt == 0), stop=(lt == NLT - 1),
                            tile_position=(0, cg), skip_group_check=True)

                for lt in range(NLT):
                    kst = K_sb[:, lt * LT:(lt + 1) * LT]
                    e_t = ep.tile([128, N], BF16, tag="E")
                    # chunk A (1536 cols, ScalarE exp + accum)
                    sA = sp.tile([128, 1536], F32, tag="S")
                    for m in range(3):
                        nc.tensor.matmul(
                            sA[:, m * NB:(m + 1) * NB], kst,
                            Qt[:, CA0 + m * NB:CA0 + (m + 1) * NB],
                            start=True, stop=True)
                    accA = stp.tile([128, 1], F32, tag="accA")
                    nc.scalar.activation(out=e_t[:, CA0:CA1], in_=sA[:, 0:1536],
                                         func=AF.Exp, accum_out=accA[:])
                    # chunk B (1024 cols, ScalarE exp + accum)
                    sB = sp.tile([128, 1536], F32, tag="S")
                    for m in range(2):
                        nc.tensor.matmul(
                            sB[:, m * NB:(m + 1) * NB], kst,
                            Qt[:, CB0 + m * NB:CB0 + (m + 1) * NB],
                            start=True, stop=True)
                    accB = stp.tile([128, 1], F32, tag="accB")
                    nc.scalar.activation(out=e_t[:, CB0:CB1], in_=sB[:, 0:1024],
                                         func=AF.Exp, accum_out=accB[:])
                    # chunk C (1536 cols, DVE bit-trick exp)
                    sC = sp.tile([128, 1536], F32, tag="S")
                    for m in range(3):
                        nc.tensor.matmul(
                            sC[:, m * NB:(m + 1) * NB], kst,
                            Qt[:, CC0 + m * NB:CC0 + (m + 1) * NB],
                            start=True, stop=True)
                    denC = stp.tile([128, 1], F32, tag="denC")
                    if FAST_EXP:
                        ec = ecp.tile([128, CCW], BF16, tag="ec")
                        nc.vector.tensor_scalar(out=ec[:].bitcast(I16),
                                                in0=sC[:, 0:CCW],
                                                scalar1=FX_SCALE,
                                                scalar2=FX_BIAS,
                                                op0=OP.mult, op1=OP.add)
                        nc.vector.tensor_scalar(out=e_t[:, CC0:CC1], in0=ec[:],
                                                scalar1=1.0, scalar2=0.0,
                                                op0=OP.mult, op1=OP.add,
                                                accum_out=denC[:])
                    else:
                        nc.scalar.activation(out=e_t[:, CC0:CC1],
                                             in_=sC[:, 0:CCW],
                                             func=AF.Exp, accum_out=denC[:])
                    # message matmuls run one tile behind: their inputs are
                    # already ready, so the PE never waits on the den chain
                    if prev is not None:
                        emit_msg(lt - 1, *prev)
                    den = stp.tile([128, 1], F32, tag="den")
                    nc.vector.scalar_tensor_tensor(
                        out=den[:], in0=accA[:], scalar=accB[:],
                        in1=denC[:], op0=OP.add, op1=OP.add)
                    rden = stp.tile([128, 1], F32, tag="rden")
                    nc.vector.reciprocal(rden[:], den[:])
                    bvr = stp.tile([128, GC], F32, tag="bvr")
                    nc.vector.tensor_scalar_mul(bvr[:], bv_bcast[:], rden[:])
                    vts = stp.tile([128, GC], BF16, tag="vts")
                    nc.vector.scalar_tensor_tensor(
                        out=vts[:], in0=Vt_raw[:, lt * GC:(lt + 1) * GC],
                        scalar=rden[:], in1=bvr[:], op0=OP.mult, op1=OP.add)
                    prev = (vts, e_t)
                emit_msg(NLT - 1, *prev)

                # unpack message strips to [32, 4096]; split DVE/ScalarE
                for j in range(NNB):
                    cg = 32 * (j % 4)
                    hb = (j // 4) * NB
                    src = msg_ps[cg:cg + 32, hb:hb + NB]
                    dst = msg_sb[:, j * NB:(j + 1) * NB]
                    if j % 2 == 0:
                        nc.vector.tensor_copy(dst, src)
                    else:
                        nc.scalar.copy(dst, src)

            # ---- tail: Wc projection + residual --------------------------
            with tc.tile_pool(name="tail_psum", bufs=2,
                              space=bass.MemorySpace.PSUM) as tp:
                for j in range(NNB):
                    blk = slice(j * NB, (j + 1) * NB)
                    pj = tp.tile([GC, NB], F32, tag="prj")
                    nc.tensor.matmul(pj[:], wkv[0:32, 128:160],
                                     msg_sb[:, blk],
                                     start=True, stop=False)
                    nc.tensor.matmul(pj[:], wkv[0:32, 192:224], graphTb[:, blk],
                                     start=False, stop=True)
                    if j % 2 == 0:
                        nc.scalar.activation(out=outT[:, blk], in_=pj[:],
                                             func=AF.Identity, bias=bc,
                                             scale=1.0)
                    else:
                        nc.vector.tensor_scalar_add(outT[:, blk], pj[:], bc)
                    if j == 3:
                        nc.sync.dma_start(out=out_d[:, 0:2048],
                                          in_=outT[:, 0:2048])
                nc.sync.dma_start(out=out_d[:, 2048:4096],
                                  in_=outT[:, 2048:4096])

    nc.finalize()
    return nc


def _get_nc():
    if "nc" not in _NC_CACHE:
        _NC_CACHE["nc"] = build_kernel()
    return _NC_CACHE["nc"]


def kernel(**inputs):
    global LAST_RESULT
    graph = np.ascontiguousarray(np.asarray(inputs["input_graph"], np.float32))
    img = np.asarray(inputs["input_image"], np.float32).reshape(B, C, L)
    Wq = np.asarray(inputs["Wq"], np.float32)
    bq = np.asarray(inputs["bq"], np.float32)
    Wk = np.asarray(inputs["Wk"], np.float32)
    bk = np.asarray(inputs["bk"], np.float32)
    Wv = np.asarray(inputs["Wv"], np.float32)
    bv = np.asarray(inputs["bv"], np.float32)
    Wc = np.asarray(inputs["Wc"], np.float32)
    bc = np.asarray(inputs["bc"], np.float32)

    s = 1.0 / np.sqrt(np.float32(GC))

    # image: [B, 256, L] -> [B, 128, 2L] (channel halves side by side), bf16
    img_b = np.ascontiguousarray(
        img.reshape(B, 2, 128, L).transpose(0, 2, 1, 3).reshape(B, 128, 2 * L)
    ).astype(ml_dtypes.bfloat16)
    graphTb = np.ascontiguousarray(
        graph.transpose(0, 2, 1)).astype(ml_dtypes.bfloat16)

    wkv = np.zeros((128, 224), np.float32)
    wkv[:, 0:32] = Wk.T[0:128]
    wkv[:, 32:64] = Wk.T[128:256]
    wkv[:, 64:96] = Wv.T[0:128]
    wkv[:, 96:128] = Wv.T[128:256]
    wkv[:, 128:160] = np.tile(Wc.T, (4, 1))
    wkv[0:32, 160:192] = Wq.T * s
    wkv[0:32, 192:224] = np.eye(32, dtype=np.float32)
    wkv = wkv.astype(ml_dtypes.bfloat16)

    wq = np.zeros((GC, 40), np.float32)
    wq[:, 0] = bq * s
    wq[:, 1] = bk
    wq[:, 2] = bc
    wq[0, 8:40] = bv

    nc = _get_nc()
    in_maps = [
        {"img": img_b[i], "graphTb": graphTb[i], "wkv": wkv, "wq": wq}
        for i in range(B)
    ]
    res = run_bass_kernel_spmd(nc, in_maps, core_ids=list(range(B)),
                               trace=TRACE)
    LAST_RESULT = res
    outT = np.stack([np.asarray(res.results[i]["outT"]) for i in range(B)])
    return np.ascontiguousarray(outT.transpose(0, 2, 1)).astype(np.float32)


# revision 12
# speedup vs baseline: 1.1867x; 1.1867x over previous
"""Trainium2 Bass kernel for nn_ConnectionC2G (GNN cross-attention message passing).

Math (per batch b):
    K = Wk @ img + bk            [32, L]   (img = image reshaped [256, L], L = 4096)
    V = Wv @ img + bv            [32, L]
    Qt = (Wq @ graph^T + bq)/s   [32, N]   (s = sqrt(32); scale folded into Wq, bq)
    S^T[l, n] = sum_o K[o,l] Qt[o,n]       (attention scores, transposed layout)
    softmax over n-axis of the ORIGINAL layout == per-l-row softmax in S^T layout
    message[o, n] = sum_l ((V[o,l]+bv)/den[l]) * exp(S^T[l,n])
    out^T = graph^T + Wc @ message + bc    [32, N]

Key tricks:
  - scores lie in [-2.6, 2.7] for this problem so exp() never overflows ->
    no max-subtraction pass.
  - HAM warm-up: the PE clock gate sits at 1.2 GHz until ~3.4us of continuous
    matmul activity.  The prologue runs the Q projections back-to-back as soon
    as graphT lands so the whole main loop runs at the 2.4 GHz warm clock.
  - exp is split across two engines: chunks A (1536 cols) and B (1024) go
    through ScalarE's table exp (accum_out gives their denominator parts),
    chunk C (1536) is computed on the DVE with the 2^y bit trick:
    bf16bits(e^s) ~= int16(s * 128*log2(e) + 127*128).  The C-chunk partial
    denominator rides along as accum_out of the bf16 copy pass.
  - 1/den and the V bias are folded into the per-tile [128, 32] vts tile, not
    the big [L, N] matrix.
  - message accumulates across all 32 l-tiles into 2 persistent PSUM banks
    using tile_position column strips; the strips are unpacked with ONE
    [128, 1024] DVE copy and consumed by Wc matmuls via tile_position row
    offsets.  The residual (graph^T) is added on the PE with an identity
    matmul; bc is folded into the PSUM->SBUF copies' bias.
  - sharding: data-parallel over batch, 1 batch per NeuronCore (8 cores).
"""

import numpy as np
import ml_dtypes

import concourse.bass as bass
import concourse.bacc as bacc
import concourse.tile as tile
from concourse import mybir
from concourse.bass_utils import run_bass_kernel_spmd

F32 = mybir.dt.float32
BF16 = mybir.dt.bfloat16
I16 = mybir.dt.int16
AF = mybir.ActivationFunctionType
OP = mybir.AluOpType

B = 8
N = 4096          # graph nodes
GC = 32           # graph channels
C = 256           # image channels
L = 4096          # image pixels (64*64)
LT = 128          # l-tile rows (partition dim of S^T tiles)
NLT = L // LT     # 32 l-tiles
NB = 512          # matmul moving-dim block
NNB = N // NB     # 8 n-blocks

# exp chunks: A, B on ScalarE (table exp + accum), C on DVE (bit-trick exp)
CA0, CA1 = 0, 1536
CB0, CB1 = 1536, 3072
CC0, CC1 = 3072, 4096
CCW = CC1 - CC0

# fast-exp constants: bf16 bits of 2^y = y*128 + 127*128, y = s*log2(e)
FX_SCALE = 128.0 * 1.4426950408889634
FX_BIAS = 127.0 * 128.0

FAST_EXP = True          # chunk C on DVE bit-trick exp (False: ScalarE exp)

TRACE = False            # test.py sets kernel.TRACE = True for profiling
LAST_RESULT = None       # test.py reads exec_time_ns from here

_NC_CACHE = {}


def _pair_ap(t, j):
    """AP covering cols [j*NB, (j+1)*NB) of BOTH channel halves of img."""
    base = t[:, j * NB:(j + 1) * NB]
    ap = list(base.ap)
    return bass.AP(tensor=base.tensor, offset=base.offset,
                   ap=[ap[0], [L, 2]] + ap[1:])


def build_kernel():
    nc = bacc.Bacc("TRN2")

    img_d = nc.dram_tensor("img", [128, 2 * L], BF16, kind="ExternalInput")
    graphTb_d = nc.dram_tensor("graphTb", [GC, N], BF16, kind="ExternalInput")
    # bf16 pack: [:,0:32] WkT rows 0:128 | [:,32:64] WkT rows 128:256
    #            [:,64:96] WvT rows 0:128 | [:,96:128] WvT rows 128:256
    #            [:,128:160] WcT tiled x4 | [0:32,160:192] WqT*s
    #            [0:32,192:224] I32 identity
    wkv_d = nc.dram_tensor("wkv", [128, 224], BF16, kind="ExternalInput")
    # f32 pack: [:,0] bq*s | [:,1] bk | [:,2] bc ; row 0 cols 8:40 = bv
    wq_d = nc.dram_tensor("wq", [GC, 40], F32, kind="ExternalInput")
    out_d = nc.dram_tensor("outT", [GC, N], F32, kind="ExternalOutput")

    with tile.TileContext(nc) as tc:
        with tc.tile_pool(name="persist", bufs=1) as persist:
            img = persist.tile([128, 2 * L], BF16, tag="img")
            graphTb = persist.tile([GC, N], BF16, tag="graphTb")
            wkv = persist.tile([128, 224], BF16, tag="wkv")
            wq = persist.tile([GC, 40], F32, tag="wq")
            bv_bcast = persist.tile([128, GC], F32, tag="bv_bcast")
            K_sb = persist.tile([GC, N], BF16, tag="K_sb")
            Qt = persist.tile([GC, N], BF16, tag="Qt")
            Vt_raw = persist.tile([128, NLT * GC], BF16, tag="Vt_raw")
            msg_sb = persist.tile([GC, N], BF16, tag="msg_sb")
            outT = persist.tile([GC, N], F32, tag="outT")

            # small tensors first (unblock Q projections), then the image as
            # 8 block-pairs spread over the two HW DGE queues (+1 on gpsimd)
            nc.scalar.dma_start(out=wkv[:], in_=wkv_d[:])
            nc.scalar.dma_start(out=wq[:], in_=wq_d[:])
            bv_row = wq_d[0:1, 8:40]
            nc.scalar.dma_start(
                out=bv_bcast[:],
                in_=bass.AP(tensor=bv_row.tensor, offset=bv_row.offset,
                            ap=[[0, 128]] + list(bv_row.ap[1:])))
            nc.scalar.dma_start(out=graphTb[:], in_=graphTb_d[:])
            # 4 col-windows x 2 halves as plain 2D DMAs; sync carries half 0,
            # scalar half 1, so window w (K blocks 2w, 2w+1) lands early.
            WW = 1024
            for w in range(4):
                cs = slice(w * WW, (w + 1) * WW)
                nc.sync.dma_start(out=img[:, cs], in_=img_d[:, cs])
                cs1 = slice(L + w * WW, L + (w + 1) * WW)
                nc.scalar.dma_start(out=img[:, cs1], in_=img_d[:, cs1])

            bq = wq[:, 0:1]
            bk = wq[:, 1:2]
            bc = wq[:, 2:3]

            # ---- prologue ------------------------------------------------
            with (
                tc.tile_pool(name="proj_psum", bufs=4,
                             space=bass.MemorySpace.PSUM) as pp,
                tc.tile_pool(name="vt_psum", bufs=4,
                             space=bass.MemorySpace.PSUM) as vtp,
            ):
                # Q projections first: 8 dense matmuls warm the PE HAM gate
                for j in range(NNB):
                    blk = slice(j * NB, (j + 1) * NB)
                    qp = pp.tile([GC, NB], F32, tag="proj")
                    nc.tensor.matmul(qp[:], wkv[0:32, 160:192], graphTb[:, blk],
                                     start=True, stop=True)
                    nc.scalar.activation(out=Qt[:, blk], in_=qp[:],
                                         func=AF.Identity, bias=bq, scale=1.0)
                # per image block: K projection + V^T l-tiles (chases DMA)
                for j in range(NNB):
                    blk = slice(j * NB, (j + 1) * NB)
                    kp = pp.tile([GC, NB], F32, tag="proj")
                    nc.tensor.matmul(kp[:], wkv[:, 0:32], img[:, blk],
                                     start=True, stop=False)
                    nc.tensor.matmul(kp[:], wkv[:, 32:64],
                                     img[:, L + j * NB:L + (j + 1) * NB],
                                     start=False, stop=True)
                    nc.vector.tensor_scalar_add(K_sb[:, blk], kp[:], bk)
                    # V^T tiles: each in its own full PSUM bank (matmul psum
                    # writes must start bank-aligned); copies split DVE/ScalarE
                    for i in range(4):
                        lt = 4 * j + i
                        vt = vtp.tile([128, NB], F32, tag="vt")
                        nc.tensor.matmul(vt[:, 0:GC],
                                         img[:, lt * LT:(lt + 1) * LT],
                                         wkv[:, 64:96], start=True, stop=False)
                        nc.tensor.matmul(vt[:, 0:GC],
                                         img[:, L + lt * LT:L + (lt + 1) * LT],
                                         wkv[:, 96:128], start=False, stop=True)
                        dst = Vt_raw[:, lt * GC:(lt + 1) * GC]
                        if lt % 2 == 0:
                            nc.vector.tensor_copy(dst, vt[:, 0:GC])
                        else:
                            nc.scalar.copy(dst, vt[:, 0:GC])

            # ---- main loop: scores -> exp -> message ---------------------
            with (
                tc.tile_pool(name="s_psum", bufs=2,
                             space=bass.MemorySpace.PSUM) as sp,
                tc.tile_pool(name="msg_psum", bufs=1,
                             space=bass.MemorySpace.PSUM) as mp,
                tc.tile_pool(name="e_pool", bufs=3) as ep,
                tc.tile_pool(name="ec_pool", bufs=2) as ecp,
                tc.tile_pool(name="stat", bufs=8) as stp,
            ):
                msg_ps = mp.tile([128, 1024], F32, tag="msg")
                prev = None  # (vts, e_t) of tile lt-1, msg emitted one behind

                def emit_msg(lt, vts, e_t):
                    for j in range(NNB):
                        cg = 32 * (j % 4)
                        hb = (j // 4) * NB
                        nc.tensor.matmul(
                            msg_ps[cg:cg + 32, hb:hb + NB],
                            vts[:], e_t[:, j * NB:(j + 1) * NB],
                            start=(lt == 0), stop=(lt == NLT - 1),
                            tile_position=(0, cg), skip_group_check=True)

                for lt in range(NLT):
                    kst = K_sb[:, lt * LT:(lt + 1) * LT]
                    e_t = ep.tile([128, N], BF16, tag="E")
                    # chunk A (1536 cols, ScalarE exp + accum)
                    sA = sp.tile([128, 1536], F32, tag="S")
                    for m in range(3):
                        nc.tensor.matmul(
                            sA[:, m * NB:(m + 1) * NB], kst,
                            Qt[:, CA0 + m * NB:CA0 + (m + 1) * NB],
                            start=True, stop=True)
                    accA = stp.tile([128, 1], F32, tag="accA")
                    nc.scalar.activation(out=e_t[:, CA0:CA1], in_=sA[:, 0:1536],
                                         func=AF.Exp, accum_out=accA[:])
                    # chunk B (1024 cols, ScalarE exp + accum)
                    sB = sp.tile([128, 1536], F32, tag="S")
                    for m in range(3):
                        nc.tensor.matmul(
                            sB[:, m * NB:(m + 1) * NB], kst,
                            Qt[:, CB0 + m * NB:CB0 + (m + 1) * NB],
                            start=True, stop=True)
                    accB = stp.tile([128, 1], F32, tag="accB")
                    nc.scalar.activation(out=e_t[:, CB0:CB1], in_=sB[:, 0:1536],
                                         func=AF.Exp, accum_out=accB[:])
                    # chunk C (1536 cols, DVE bit-trick exp)
                    sC = sp.tile([128, 1536], F32, tag="S")
                    for m in range(2):
                        nc.tensor.matmul(
                            sC[:, m * NB:(m + 1) * NB], kst,
                            Qt[:, CC0 + m * NB:CC0 + (m + 1) * NB],
                            start=True, stop=True)
                    denC = stp.tile([128, 1], F32, tag="denC")
                    if FAST_EXP:
                        ec = ecp.tile([128, CCW], BF16, tag="ec")
                        nc.vector.tensor_scalar(out=ec[:].bitcast(I16),
                                                in0=sC[:, 0:CCW],
                                                scalar1=FX_SCALE,
                                                scalar2=FX_BIAS,
                                                op0=OP.mult, op1=OP.add)
                        nc.vector.tensor_scalar(out=e_t[:, CC0:CC1], in0=ec[:],
                                                scalar1=1.0, scalar2=0.0,
                                                op0=OP.mult, op1=OP.add,
                                                accum_out=denC[:])
                    else:
                        nc.scalar.activation(out=e_t[:, CC0:CC1],
                                             in_=sC[:, 0:CCW],
                                             func=AF.Exp, accum_out=denC[:])
                    # message matmuls run one tile behind: their inputs are
                    # already ready, so the PE never waits on the den chain
                    if prev is not None:
                        emit_msg(lt - 1, *prev)
                    den = stp.tile([128, 1], F32, tag="den")
                    nc.vector.scalar_tensor_tensor(
                        out=den[:], in0=accA[:], scalar=accB[:],
                        in1=denC[:], op0=OP.add, op1=OP.add)
                    rden = stp.tile([128, 1], F32, tag="rden")
                    nc.vector.reciprocal(rden[:], den[:])
                    bvr = stp.tile([128, GC], F32, tag="bvr")
                    nc.vector.tensor_scalar_mul(bvr[:], bv_bcast[:], rden[:])
                    vts = stp.tile([128, GC], BF16, tag="vts")
                    nc.vector.scalar_tensor_tensor(
                        out=vts[:], in0=Vt_raw[:, lt * GC:(lt + 1) * GC],
                        scalar=rden[:], in1=bvr[:], op0=OP.mult, op1=OP.add)
                    prev = (vts, e_t)
                emit_msg(NLT - 1, *prev)

                # unpack message strips to [32, 4096]; split DVE/ScalarE
                for j in range(NNB):
                    cg = 32 * (j % 4)
                    hb = (j // 4) * NB
                    src = msg_ps[cg:cg + 32, hb:hb + NB]
                    dst = msg_sb[:, j * NB:(j + 1) * NB]
                    if j % 2 == 0:
                        nc.vector.tensor_copy(dst, src)
                    else:
                        nc.scalar.copy(dst, src)

            # ---- tail: Wc projection + residual --------------------------
            with tc.tile_pool(name="tail_psum", bufs=2,
                              space=bass.MemorySpace.PSUM) as tp:
                for j in range(NNB):
                    blk = slice(j * NB, (j + 1) * NB)
                    pj = tp.tile([GC, NB], F32, tag="prj")
                    nc.tensor.matmul(pj[:], wkv[0:32, 128:160],
                                     msg_sb[:, blk],
                                     start=True, stop=False)
                    nc.tensor.matmul(pj[:], wkv[0:32, 192:224], graphTb[:, blk],
                                     start=False, stop=True)
                    if j % 2 == 0:
                        nc.scalar.activation(out=outT[:, blk], in_=pj[:],
                                             func=AF.Identity, bias=bc,
                                             scale=1.0)
                    else:
                        nc.vector.tensor_scalar_add(outT[:, blk], pj[:], bc)
                    if j == 3:
                        nc.sync.dma_start(out=out_d[:, 0:2048],
                                          in_=outT[:, 0:2048])
                nc.sync.dma_start(out=out_d[:, 2048:4096],
                                  in_=outT[:, 2048:4096])

    nc.finalize()
    return nc


def _get_nc():
    if "nc" not in _NC_CACHE:
        _NC_CACHE["nc"] = build_kernel()
    return _NC_CACHE["nc"]


def kernel(**inputs):
    global LAST_RESULT
    graph = np.ascontiguousarray(np.asarray(inputs["input_graph"], np.float32))
    img = np.asarray(inputs["input_image"], np.float32).reshape(B, C, L)
    Wq = np.asarray(inputs["Wq"], np.float32)
    bq = np.asarray(inputs["bq"], np.float32)
    Wk = np.asarray(inputs["Wk"], np.float32)
    bk = np.asarray(inputs["bk"], np.float32)
    Wv = np.asarray(inputs["Wv"], np.float32)
    bv = np.asarray(inputs["bv"], np.float32)
    Wc = np.asarray(inputs["Wc"], np.float32)
    bc = np.asarray(inputs["bc"], np.float32)

    s = 1.0 / np.sqrt(np.float32(GC))

    # image: [B, 256, L] -> [B, 128, 2L] (channel halves side by side), bf16
    img_b = np.ascontiguousarray(
        img.reshape(B, 2, 128, L).transpose(0, 2, 1, 3).reshape(B, 128, 2 * L)
    ).astype(ml_dtypes.bfloat16)
    graphTb = np.ascontiguousarray(
        graph.transpose(0, 2, 1)).astype(ml_dtypes.bfloat16)

    wkv = np.zeros((128, 224), np.float32)
    wkv[:, 0:32] = Wk.T[0:128]
    wkv[:, 32:64] = Wk.T[128:256]
    wkv[:, 64:96] = Wv.T[0:128]
    wkv[:, 96:128] = Wv.T[128:256]
    wkv[:, 128:160] = np.tile(Wc.T, (4, 1))
    wkv[0:32, 160:192] = Wq.T * s
    wkv[0:32, 192:224] = np.eye(32, dtype=np.float32)
    wkv = wkv.astype(ml_dtypes.bfloat16)

    wq = np.zeros((GC, 40), np.float32)
    wq[:, 0] = bq * s
    wq[:, 1] = bk
    wq[:, 2] = bc
    wq[0, 8:40] = bv

    nc = _get_nc()
    in_maps = [
        {"img": img_b[i], "graphTb": graphTb[i], "wkv": wkv, "wq": wq}
        for i in range(B)
    ]
    res = run_bass_kernel_spmd(nc, in_maps, core_ids=list(range(B)),
                               trace=TRACE)
    LAST_RESULT = res
    outT = np.stack([np.asarray(res.results[i]["outT"]) for i in range(B)])
    return np.ascontiguousarray(outT.transpose(0, 2, 1)).astype(np.float32)


# revision 13
# speedup vs baseline: 1.2766x; 1.0757x over previous
"""Trainium2 Bass kernel for nn_ConnectionC2G (GNN cross-attention message passing).

Math (per batch b):
    K = Wk @ img + bk            [32, L]   (img = image reshaped [256, L], L = 4096)
    V = Wv @ img + bv            [32, L]
    Qt = (Wq @ graph^T + bq)/s   [32, N]   (s = sqrt(32); scale folded into Wq, bq)
    S^T[l, n] = sum_o K[o,l] Qt[o,n]       (attention scores, transposed layout)
    softmax over n-axis of the ORIGINAL layout == per-l-row softmax in S^T layout
    message[o, n] = sum_l ((V[o,l]+bv)/den[l]) * exp(S^T[l,n])
    out^T = graph^T + Wc @ message + bc    [32, N]

Key tricks:
  - scores lie in [-2.6, 2.7] for this problem so exp() never overflows ->
    no max-subtraction pass.
  - HAM warm-up: the PE clock gate sits at 1.2 GHz until ~3.4us of continuous
    matmul activity.  The prologue runs the Q projections back-to-back as soon
    as graphT lands so the whole main loop runs at the 2.4 GHz warm clock.
  - exp is split across two engines: chunks A (1536 cols) and B (1024) go
    through ScalarE's table exp (accum_out gives their denominator parts),
    chunk C (1536) is computed on the DVE with the 2^y bit trick:
    bf16bits(e^s) ~= int16(s * 128*log2(e) + 127*128).  The C-chunk partial
    denominator rides along as accum_out of the bf16 copy pass.
  - 1/den and the V bias are folded into the per-tile [128, 32] vts tile, not
    the big [L, N] matrix.
  - message accumulates across all 32 l-tiles into 2 persistent PSUM banks
    using tile_position column strips; the strips are unpacked with ONE
    [128, 1024] DVE copy and consumed by Wc matmuls via tile_position row
    offsets.  The residual (graph^T) is added on the PE with an identity
    matmul; bc is folded into the PSUM->SBUF copies' bias.
  - sharding: data-parallel over batch, 1 batch per NeuronCore (8 cores).
"""

import numpy as np
import ml_dtypes

import concourse.bass as bass
import concourse.bacc as bacc
import concourse.tile as tile
from concourse import mybir
from concourse.bass_utils import run_bass_kernel_spmd

F32 = mybir.dt.float32
BF16 = mybir.dt.bfloat16
I16 = mybir.dt.int16
AF = mybir.ActivationFunctionType
OP = mybir.AluOpType

B = 8
N = 4096          # graph nodes
GC = 32           # graph channels
C = 256           # image channels
L = 4096          # image pixels (64*64)
LT = 128          # l-tile rows (partition dim of S^T tiles)
NLT = L // LT     # 32 l-tiles
NB = 512          # matmul moving-dim block
NNB = N // NB     # 8 n-blocks

# exp chunks: A, B on ScalarE (table exp + accum), C on DVE (bit-trick exp)
CA0, CA1 = 0, 1536
CB0, CB1 = 1536, 3072
CC0, CC1 = 3072, 4096
CCW = CC1 - CC0

# fast-exp constants: bf16 bits of 2^y = y*128 + 127*128, y = s*log2(e)
FX_SCALE = 128.0 * 1.4426950408889634
FX_BIAS = 127.0 * 128.0

FAST_EXP = False         # chunk C on DVE bit-trick exp (False: ScalarE exp)

TRACE = False            # test.py sets kernel.TRACE = True for profiling
LAST_RESULT = None       # test.py reads exec_time_ns from here

_NC_CACHE = {}


def _pair_ap(t, j):
    """AP covering cols [j*NB, (j+1)*NB) of BOTH channel halves of img."""
    base = t[:, j * NB:(j + 1) * NB]
    ap = list(base.ap)
    return bass.AP(tensor=base.tensor, offset=base.offset,
                   ap=[ap[0], [L, 2]] + ap[1:])


def build_kernel():
    nc = bacc.Bacc("TRN2")

    img_d = nc.dram_tensor("img", [128, 2 * L], BF16, kind="ExternalInput")
    graphTb_d = nc.dram_tensor("graphTb", [GC, N], BF16, kind="ExternalInput")
    # bf16 pack: [:,0:32] WkT rows 0:128 | [:,32:64] WkT rows 128:256
    #            [:,64:96] WvT rows 0:128 | [:,96:128] WvT rows 128:256
    #            [:,128:160] WcT tiled x4 | [0:32,160:192] WqT*s
    #            [0:32,192:224] I32 identity
    wkv_d = nc.dram_tensor("wkv", [128, 224], BF16, kind="ExternalInput")
    # f32 pack: [:,0] bq*s | [:,1] bk | [:,2] bc ; row 0 cols 8:40 = bv
    wq_d = nc.dram_tensor("wq", [GC, 40], F32, kind="ExternalInput")
    out_d = nc.dram_tensor("outT", [GC, N], F32, kind="ExternalOutput")

    with tile.TileContext(nc) as tc:
        with tc.tile_pool(name="persist", bufs=1) as persist:
            img = persist.tile([128, 2 * L], BF16, tag="img")
            graphTb = persist.tile([GC, N], BF16, tag="graphTb")
            wkv = persist.tile([128, 224], BF16, tag="wkv")
            wq = persist.tile([GC, 40], F32, tag="wq")
            bv_bcast = persist.tile([128, GC], F32, tag="bv_bcast")
            K_sb = persist.tile([GC, N], BF16, tag="K_sb")
            Qt = persist.tile([GC, N], BF16, tag="Qt")
            Vt_raw = persist.tile([128, NLT * GC], BF16, tag="Vt_raw")
            msg_sb = persist.tile([GC, N], BF16, tag="msg_sb")
            outT = persist.tile([GC, N], F32, tag="outT")

            # small tensors first (unblock Q projections), then the image as
            # 8 block-pairs spread over the two HW DGE queues (+1 on gpsimd)
            nc.scalar.dma_start(out=wkv[:], in_=wkv_d[:])
            nc.scalar.dma_start(out=wq[:], in_=wq_d[:])
            bv_row = wq_d[0:1, 8:40]
            nc.scalar.dma_start(
                out=bv_bcast[:],
                in_=bass.AP(tensor=bv_row.tensor, offset=bv_row.offset,
                            ap=[[0, 128]] + list(bv_row.ap[1:])))
            nc.scalar.dma_start(out=graphTb[:], in_=graphTb_d[:])
            # 4 col-windows x 2 halves as plain 2D DMAs; sync carries half 0,
            # scalar half 1, so window w (K blocks 2w, 2w+1) lands early.
            WW = 1024
            for w in range(4):
                cs = slice(w * WW, (w + 1) * WW)
                nc.sync.dma_start(out=img[:, cs], in_=img_d[:, cs])
                cs1 = slice(L + w * WW, L + (w + 1) * WW)
                nc.scalar.dma_start(out=img[:, cs1], in_=img_d[:, cs1])

            bq = wq[:, 0:1]
            bk = wq[:, 1:2]
            bc = wq[:, 2:3]

            # ---- prologue ------------------------------------------------
            with (
                tc.tile_pool(name="proj_psum", bufs=4,
                             space=bass.MemorySpace.PSUM) as pp,
                tc.tile_pool(name="vt_psum", bufs=4,
                             space=bass.MemorySpace.PSUM) as vtp,
            ):
                # Q projections first: 8 dense matmuls warm the PE HAM gate
                for j in range(NNB):
                    blk = slice(j * NB, (j + 1) * NB)
                    qp = pp.tile([GC, NB], F32, tag="proj")
                    nc.tensor.matmul(qp[:], wkv[0:32, 160:192], graphTb[:, blk],
                                     start=True, stop=True)
                    nc.scalar.activation(out=Qt[:, blk], in_=qp[:],
                                         func=AF.Identity, bias=bq, scale=1.0)
                # per image block: K projection + V^T l-tiles (chases DMA)
                for j in range(NNB):
                    blk = slice(j * NB, (j + 1) * NB)
                    kp = pp.tile([GC, NB], F32, tag="proj")
                    nc.tensor.matmul(kp[:], wkv[:, 0:32], img[:, blk],
                                     start=True, stop=False)
                    nc.tensor.matmul(kp[:], wkv[:, 32:64],
                                     img[:, L + j * NB:L + (j + 1) * NB],
                                     start=False, stop=True)
                    nc.vector.tensor_scalar_add(K_sb[:, blk], kp[:], bk)
                    # V^T tiles: each in its own full PSUM bank (matmul psum
                    # writes must start bank-aligned); copies split DVE/ScalarE
                    for i in range(4):
                        lt = 4 * j + i
                        vt = vtp.tile([128, NB], F32, tag="vt")
                        nc.tensor.matmul(vt[:, 0:GC],
                                         img[:, lt * LT:(lt + 1) * LT],
                                         wkv[:, 64:96], start=True, stop=False)
                        nc.tensor.matmul(vt[:, 0:GC],
                                         img[:, L + lt * LT:L + (lt + 1) * LT],
                                         wkv[:, 96:128], start=False, stop=True)
                        dst = Vt_raw[:, lt * GC:(lt + 1) * GC]
                        if lt % 2 == 0:
                            nc.vector.tensor_copy(dst, vt[:, 0:GC])
                        else:
                            nc.scalar.copy(dst, vt[:, 0:GC])

            # ---- main loop: scores -> exp -> message ---------------------
            with (
                tc.tile_pool(name="s_psum", bufs=2,
                             space=bass.MemorySpace.PSUM) as sp,
                tc.tile_pool(name="msg_psum", bufs=1,
                             space=bass.MemorySpace.PSUM) as mp,
                tc.tile_pool(name="e_pool", bufs=3) as ep,
                tc.tile_pool(name="ec_pool", bufs=2) as ecp,
                tc.tile_pool(name="stat", bufs=8) as stp,
            ):
                msg_ps = mp.tile([128, 1024], F32, tag="msg")
                prev = None  # (vts, e_t) of tile lt-1, msg emitted one behind

                def emit_msg(lt, vts, e_t):
                    for j in range(NNB):
                        cg = 32 * (j % 4)
                        hb = (j // 4) * NB
                        nc.tensor.matmul(
                            msg_ps[cg:cg + 32, hb:hb + NB],
                            vts[:], e_t[:, j * NB:(j + 1) * NB],
                            start=(lt == 0), stop=(lt == NLT - 1),
                            tile_position=(0, cg), skip_group_check=True)

                for lt in range(NLT):
                    kst = K_sb[:, lt * LT:(lt + 1) * LT]
                    e_t = ep.tile([128, N], BF16, tag="E")
                    # chunk A (1536 cols, ScalarE exp + accum)
                    sA = sp.tile([128, 1536], F32, tag="S")
                    for m in range(3):
                        nc.tensor.matmul(
                            sA[:, m * NB:(m + 1) * NB], kst,
                            Qt[:, CA0 + m * NB:CA0 + (m + 1) * NB],
                            start=True, stop=True)
                    accA = stp.tile([128, 1], F32, tag="accA")
                    nc.scalar.activation(out=e_t[:, CA0:CA1], in_=sA[:, 0:1536],
                                         func=AF.Exp, accum_out=accA[:])
                    # chunk B (1024 cols, ScalarE exp + accum)
                    sB = sp.tile([128, 1536], F32, tag="S")
                    for m in range(3):
                        nc.tensor.matmul(
                            sB[:, m * NB:(m + 1) * NB], kst,
                            Qt[:, CB0 + m * NB:CB0 + (m + 1) * NB],
                            start=True, stop=True)
                    accB = stp.tile([128, 1], F32, tag="accB")
                    nc.scalar.activation(out=e_t[:, CB0:CB1], in_=sB[:, 0:1536],
                                         func=AF.Exp, accum_out=accB[:])
                    # chunk C (1536 cols, DVE bit-trick exp)
                    sC = sp.tile([128, 1536], F32, tag="S")
                    for m in range(2):
                        nc.tensor.matmul(
                            sC[:, m * NB:(m + 1) * NB], kst,
                            Qt[:, CC0 + m * NB:CC0 + (m + 1) * NB],
                            start=True, stop=True)
                    denC = stp.tile([128, 1], F32, tag="denC")
                    if FAST_EXP:
                        ec = ecp.tile([128, CCW], BF16, tag="ec")
                        nc.vector.tensor_scalar(out=ec[:].bitcast(I16),
                                                in0=sC[:, 0:CCW],
                                                scalar1=FX_SCALE,
                                                scalar2=FX_BIAS,
                                                op0=OP.mult, op1=OP.add)
                        nc.vector.tensor_scalar(out=e_t[:, CC0:CC1], in0=ec[:],
                                                scalar1=1.0, scalar2=0.0,
                                                op0=OP.mult, op1=OP.add,
                                                accum_out=denC[:])
                    else:
                        nc.scalar.activation(out=e_t[:, CC0:CC1],
                                             in_=sC[:, 0:CCW],
                                             func=AF.Exp, accum_out=denC[:])
                    # message matmuls run one tile behind: their inputs are
                    # already ready, so the PE never waits on the den chain
                    if prev is not None:
                        emit_msg(lt - 1, *prev)
                    den = stp.tile([128, 1], F32, tag="den")
                    nc.vector.scalar_tensor_tensor(
                        out=den[:], in0=accA[:], scalar=accB[:],
                        in1=denC[:], op0=OP.add, op1=OP.add)
                    rden = stp.tile([128, 1], F32, tag="rden")
                    nc.vector.reciprocal(rden[:], den[:])
                    bvr = stp.tile([128, GC], F32, tag="bvr")
                    nc.vector.tensor_scalar_mul(bvr[:], bv_bcast[:], rden[:])
                    vts = stp.tile([128, GC], BF16, tag="vts")
                    nc.vector.scalar_tensor_tensor(
                        out=vts[:], in0=Vt_raw[:, lt * GC:(lt + 1) * GC],
                        scalar=rden[:], in1=bvr[:], op0=OP.mult, op1=OP.add)
                    prev = (vts, e_t)
                emit_msg(NLT - 1, *prev)

                # unpack message strips to [32, 4096]; split DVE/ScalarE
                for j in range(NNB):
                    cg = 32 * (j % 4)
                    hb = (j // 4) * NB
                    src = msg_ps[cg:cg + 32, hb:hb + NB]
                    dst = msg_sb[:, j * NB:(j + 1) * NB]
                    if j % 2 == 0:
                        nc.vector.tensor_copy(dst, src)
                    else:
                        nc.scalar.copy(dst, src)

            # ---- tail: Wc projection + residual --------------------------
            with tc.tile_pool(name="tail_psum", bufs=2,
                              space=bass.MemorySpace.PSUM) as tp:
                for j in range(NNB):
                    blk = slice(j * NB, (j + 1) * NB)
                    pj = tp.tile([GC, NB], F32, tag="prj")
                    nc.tensor.matmul(pj[:], wkv[0:32, 128:160],
                                     msg_sb[:, blk],
                                     start=True, stop=False)
                    nc.tensor.matmul(pj[:], wkv[0:32, 192:224], graphTb[:, blk],
                                     start=False, stop=True)
                    if j % 2 == 0:
                        nc.scalar.activation(out=outT[:, blk], in_=pj[:],
                                             func=AF.Identity, bias=bc,
                                             scale=1.0)
                    else:
                        nc.vector.tensor_scalar_add(outT[:, blk], pj[:], bc)
                    if j == 3:
                        nc.sync.dma_start(out=out_d[:, 0:2048],
                                          in_=outT[:, 0:2048])
                nc.sync.dma_start(out=out_d[:, 2048:4096],
                                  in_=outT[:, 2048:4096])

    nc.finalize()
    return nc


def _get_nc():
    if "nc" not in _NC_CACHE:
        _NC_CACHE["nc"] = build_kernel()
    return _NC_CACHE["nc"]


def kernel(**inputs):
    global LAST_RESULT
    graph = np.ascontiguousarray(np.asarray(inputs["input_graph"], np.float32))
    img = np.asarray(inputs["input_image"], np.float32).reshape(B, C, L)
    Wq = np.asarray(inputs["Wq"], np.float32)
    bq = np.asarray(inputs["bq"], np.float32)
    Wk = np.asarray(inputs["Wk"], np.float32)
    bk = np.asarray(inputs["bk"], np.float32)
    Wv = np.asarray(inputs["Wv"], np.float32)
    bv = np.asarray(inputs["bv"], np.float32)
    Wc = np.asarray(inputs["Wc"], np.float32)
    bc = np.asarray(inputs["bc"], np.float32)

    s = 1.0 / np.sqrt(np.float32(GC))

    # image: [B, 256, L] -> [B, 128, 2L] (channel halves side by side), bf16
    img_b = np.ascontiguousarray(
        img.reshape(B, 2, 128, L).transpose(0, 2, 1, 3).reshape(B, 128, 2 * L)
    ).astype(ml_dtypes.bfloat16)
    graphTb = np.ascontiguousarray(
        graph.transpose(0, 2, 1)).astype(ml_dtypes.bfloat16)

    wkv = np.zeros((128, 224), np.float32)
    wkv[:, 0:32] = Wk.T[0:128]
    wkv[:, 32:64] = Wk.T[128:256]
    wkv[:, 64:96] = Wv.T[0:128]
    wkv[:, 96:128] = Wv.T[128:256]
    wkv[:, 128:160] = np.tile(Wc.T, (4, 1))
    wkv[0:32, 160:192] = Wq.T * s
    wkv[0:32, 192:224] = np.eye(32, dtype=np.float32)
    wkv = wkv.astype(ml_dtypes.bfloat16)

    wq = np.zeros((GC, 40), np.float32)
    wq[:, 0] = bq * s
    wq[:, 1] = bk
    wq[:, 2] = bc
    wq[0, 8:40] = bv

    nc = _get_nc()
    in_maps = [
        {"img": img_b[i], "graphTb": graphTb[i], "wkv": wkv, "wq": wq}
        for i in range(B)
    ]
    res = run_bass_kernel_spmd(nc, in_maps, core_ids=list(range(B)),
                               trace=TRACE)
    LAST_RESULT = res
    outT = np.stack([np.asarray(res.results[i]["outT"]) for i in range(B)])
    return np.ascontiguousarray(outT.transpose(0, 2, 1)).astype(np.float32)
